# revision 13
# baseline (speedup 1.0000x reference)
"""Trainium2 Bass kernel for nn_PhyHGkNN4 (spectral HGalerkin NN).

Sharding: data-parallel over batch B=8 across 8 NeuronCores (one sample
per core); small weights replicated via each core's input map; no
collectives.  Self-contained: all shapes hardcoded from the spec.

Per-core layout decisions:
  - h kept (n-part, c-free) in f16 for free-axis LayerNorm; hT (c,n) f16
    via PE transposes feeds the skip-path matmuls.
  - Fourier/Gauss bases built in (M,N) layout by two 5-row fp32r matmuls
    against [g0^2,g1^2,g0,g1,1]; sin via magic-constant range reduction;
    (N,M) layout via PE transposes.  Norms + 1/N folded into tiny
    per-partition scales applied at the t/y copies (zero bulk passes).
  - Spectral path (xh->t->y->x1) in bf16 (it contributes ~3e-4 of the
    pre-LN signal, so bf16 error is negligible); skip path in f16.
"""
import sys

sys.path.insert(0, "/opt/trn_rl_repo")

import math
from contextlib import ExitStack

import numpy as np
import ml_dtypes

import concourse.bass as bass
import concourse.tile as tile
from concourse import bacc, mybir
from concourse.bass_utils import run_bass_kernel_spmd
from concourse.masks import make_identity

F32 = mybir.dt.float32
F32R = mybir.dt.float32r
BF16 = mybir.dt.bfloat16
F16 = mybir.dt.float16
AF = mybir.ActivationFunctionType
ALU = mybir.AluOpType

B, N, C, M, KM, L = 8, 8192, 128, 256, 16, 3
K1, K2 = 64, 128
NCH = N // 128
EPS = 1e-5
MAGIC = float(1.5 * 2 ** 23)
TWO_PI = float(2 * np.pi)
SQRT_N = math.sqrt(N)

bf16 = ml_dtypes.bfloat16
f16 = np.float16


def _build(affine: bool):
    nc = bacc.Bacc()

    xb_d = nc.declare_dram_parameter("xb", [4, N], F32, isOutput=False)
    fsq_d = nc.declare_dram_parameter("fsq", [128, 128], F32, isOutput=False)
    a5f_d = nc.declare_dram_parameter("a5f", [5, 128], F32, isOutput=False)
    a5g_d = nc.declare_dram_parameter("a5g", [5, 128], F32, isOutput=False)
    fc0p_d = nc.declare_dram_parameter("fc0p", [4, C], F32, isOutput=False)
    spw_d = nc.declare_dram_parameter("spw", [128, L, KM * 128], BF16, isOutput=False)
    h1t_d = nc.declare_dram_parameter("h1t", [128, 2, KM, 256], BF16, isOutput=False)
    wst_d = nc.declare_dram_parameter("wst", [128, L, C], F16, isOutput=False)
    wsb_d = nc.declare_dram_parameter("wsb", [1, L, C], F16, isOutput=False)
    fc1w_d = nc.declare_dram_parameter("fc1w", [C, C], F16, isOutput=False)
    fc1b_d = nc.declare_dram_parameter("fc1b", [1, C], F16, isOutput=False)
    fc2w_d = nc.declare_dram_parameter("fc2w", [C, 1], F16, isOutput=False)
    fc2b_d = nc.declare_dram_parameter("fc2b", [1, 1], F16, isOutput=False)
    if affine:
        lng_d = nc.declare_dram_parameter("lng", [128, L + 1, C], F32, isOutput=False)
        lnb_d = nc.declare_dram_parameter("lnb", [128, L + 1, C], F32, isOutput=False)
    out_d = nc.declare_dram_parameter("out", [N], F32, isOutput=True)
    sqs_d = nc.dram_tensor("sq_scratch", [128, 128], F32)

    with ExitStack() as ctx:
        tc = ctx.enter_context(tile.TileContext(nc))
        pp = ctx.enter_context(tc.tile_pool(name="ps", bufs=1, space="PSUM"))
        pk = ctx.enter_context(tc.tile_pool(name="persist", bufs=1))

        # ---------- persistent SBUF ----------
        basesT = pk.tile([128, 2, N], BF16)
        bases_nm = pk.tile([128, NCH, 256], BF16)
        h = pk.tile([128, NCH, 128], F16)
        hT = pk.tile([128, N], F16)
        h1t = pk.tile([128, 2, KM, 256], BF16)
        spw = pk.tile([128, L, KM * 128], BF16)
        xh_sb = pk.tile([128, 256], BF16)
        ys_f32 = pk.tile([128, 256], F32)
        y_sb = pk.tile([128, 2, 128], BF16)
        ssq = pk.tile([128, 2, 32], F32)
        ssqt = pk.tile([128, 2], F32)
        nrm = pk.tile([128, 2], F32)
        srec = pk.tile([128, 2], F32)
        scl_tn = pk.tile([128, 2], F32)
        scl_y = pk.tile([128, 2], F32)
        stats = pk.tile([128, NCH, 2], F32)
        rst = pk.tile([128, NCH], F32)
        nmr = pk.tile([128, NCH], F32)
        sc1 = pk.tile([128, NCH], F32)
        id_bf = pk.tile([128, 128], BF16)
        id_f16 = pk.tile([128, 128], F16)
        id_f32 = pk.tile([128, 128], F32)
        a5f = pk.tile([5, 128], F32)
        a5g = pk.tile([5, 128], F32)
        wst = pk.tile([128, L, C], F16)
        wsb = pk.tile([1, L, C], F16)
        fc1w = pk.tile([C, C], F16)
        fc1b = pk.tile([1, C], F16)
        fc2w = pk.tile([C, 1], F16)
        fc2b = pk.tile([1, 1], F16)
        ones_sm = pk.tile([1, 512], F16)
        epsc = pk.tile([128, 1], F32)
        jd = pk.tile([128, 128], F32)
        ja = pk.tile([128, 128], F32)
        ones128 = pk.tile([128, 128], F32)
        if affine:
            lng = pk.tile([128, L + 1, C], F32)
            lnb = pk.tile([128, L + 1, C], F32)

        nc.sync.dma_start(h1t[:, :, :, :], h1t_d[:, :, :, :])
        nc.sync.dma_start(spw[:, :, :], spw_d[:, :, :])
        nc.sync.dma_start(a5f[:, :], a5f_d[:, :])
        nc.sync.dma_start(a5g[:, :], a5g_d[:, :])
        nc.sync.dma_start(wst[:, :, :], wst_d[:, :, :])
        nc.sync.dma_start(wsb[:, :, :], wsb_d[:, :, :])
        nc.sync.dma_start(fc1w[:, :], fc1w_d[:, :])
        nc.sync.dma_start(fc1b[:, :], fc1b_d[:, :])
        nc.sync.dma_start(fc2w[:, :], fc2w_d[:, :])
        nc.sync.dma_start(fc2b[:, :], fc2b_d[:, :])
        if affine:
            nc.sync.dma_start(lng[:, :, :], lng_d[:, :, :])
            nc.sync.dma_start(lnb[:, :, :], lnb_d[:, :, :])
        nc.gpsimd.memset(ones_sm[:, :], 1.0)
        nc.gpsimd.memset(epsc[:, :], EPS)
        nc.gpsimd.memset(ones128[:, :], 1.0)
        make_identity(nc, id_bf[:, :])
        make_identity(nc, id_f16[:, :])
        make_identity(nc, id_f32[:, :])

        # ---------- LN helpers (shared state) ----------
        ln_ctx = {}

        GSZ = 8

        def ln_chunk(j, ps_ap):
            k = j % GSZ
            if k == 0:
                ln_ctx["hp"] = pk.tile([128, GSZ, 128], F32, tag="hpre", bufs=2,
                                       name="hpre")
            hp = ln_ctx["hp"]
            nc.vector.scalar_tensor_tensor(hp[:, k, :], ps_ap, 1.0, ones128[:, :],
                                           ALU.mult, ALU.mult,
                                           accum_out=stats[:, j, 0:1])
            nc.scalar.activation(ja[:, :], hp[:, k, :], AF.Square,
                                 accum_out=stats[:, j, 1:2])

        def ln_group(g, li, gelu):
            hp = ln_ctx["hp"]
            gs = slice(g * GSZ, (g + 1) * GSZ)
            nc.vector.tensor_scalar_mul(sc1[:, gs], stats[:, gs, 0], 1.0 / 128.0)
            nc.vector.tensor_tensor(rst[:, gs], sc1[:, gs], sc1[:, gs], ALU.mult)
            nc.vector.scalar_tensor_tensor(rst[:, gs], stats[:, gs, 1], 1.0 / 128.0,
                                           rst[:, gs], ALU.mult, ALU.subtract)
            nc.scalar.activation(rst[:, gs], rst[:, gs], AF.Sqrt, bias=epsc[:, 0:1])
            nc.vector.reciprocal(rst[:, gs], rst[:, gs])
            nc.vector.scalar_tensor_tensor(nmr[:, gs], sc1[:, gs], -1.0, rst[:, gs],
                                           ALU.mult, ALU.mult)
            for k in range(GSZ):
                j = g * GSZ + k
                if affine:
                    nc.scalar.activation(hp[:, k, :], hp[:, k, :], AF.Identity,
                                         scale=rst[:, j:j + 1], bias=nmr[:, j:j + 1])
                    nc.vector.tensor_tensor(hp[:, k, :], hp[:, k, :],
                                            lng[:, li, :], ALU.mult)
                    nc.vector.tensor_tensor(hp[:, k, :], hp[:, k, :],
                                            lnb[:, li, :], ALU.add)
                    fn = AF.Gelu if gelu else AF.Identity
                    nc.scalar.activation(h[:, j, :], hp[:, k, :], fn)
                else:
                    fn = AF.Gelu if gelu else AF.Identity
                    nc.scalar.activation(h[:, j, :], hp[:, k, :], fn,
                                         scale=rst[:, j:j + 1], bias=nmr[:, j:j + 1])

        # ================= bases + fc0 (scratch pool px) =================
        with tc.tile_pool(name="scratch", bufs=1) as px:
            xb = px.tile([36, N], F32)
            fc0p = px.tile([36, C], F32)
            fsq = px.tile([128, 128], F32)
            jx_d = px.tile([128, 256], F32)
            jx_a = px.tile([128, 256], F32)

            nc.sync.dma_start(xb[2:4, :], xb_d[0:2, :])
            nc.sync.dma_start(xb[32:34, :], xb_d[0:2, :])
            nc.sync.dma_start(xb[35:36, :], xb_d[2:3, :])
            nc.sync.dma_start(fsq[:, :], fsq_d[:, :])
            nc.sync.dma_start(fc0p[32:36, :], fc0p_d[:, :])
            nc.sync.dma_start(xb[4:5, :], xb_d[3:4, :])
            nc.sync.dma_start(xb[34:35, :], xb_d[3:4, :])

            nc.vector.tensor_tensor(fsq[:, :], fsq[:, :], fsq[:, :], ALU.mult)
            nc.sync.dma_start(sqs_d[:, :], fsq[:, :])
            nc.sync.dma_start(
                xb[0:2, :],
                sqs_d[:, :].rearrange("p f -> (p f)").rearrange("(a n) -> a n", a=2),
            )

            # ---- basesT (M,N): 32 chunks of 256 per tile ----
            for mt in range(2):
                a5 = a5f if mt == 0 else a5g
                for chn in range(32):
                    cs = slice(chn * 256, (chn + 1) * 256)
                    ps = pp.tile([128, 256], F32, tag="mm256", bufs=2, name="ps_b")
                    nc.tensor.matmul(ps[:, :], a5[:, :], xb[0:5, cs],
                                     start=True, stop=True)
                    bdst = basesT[:, mt, cs]
                    if mt == 0:
                        u1 = px.tile([128, 256], F32, tag="u1", bufs=2, name="u1")
                        nc.vector.tensor_scalar(u1[:, :], ps[:, :], 1.0 / TWO_PI,
                                                MAGIC, ALU.mult, ALU.add)
                        u2 = px.tile([128, 256], F32, tag="u2", bufs=2, name="u2")
                        nc.gpsimd.tensor_scalar_sub(u2[:, :], u1[:, :], MAGIC)
                        r1 = px.tile([128, 256], F32, tag="r1", bufs=2, name="r1")
                        nc.vector.scalar_tensor_tensor(r1[:, :], u2[:, :], -TWO_PI,
                                                       ps[:, :], ALU.mult, ALU.add)
                        nc.scalar.activation(bdst, r1[:, :], AF.Sin)
                        nc.scalar.activation(jx_d[:, :], bdst, AF.Square,
                                             accum_out=ssq[:, 0, chn:chn + 1])
                    else:
                        nc.scalar.activation(bdst, ps[:, :], AF.Exp)
                        nc.scalar.activation(jx_a[:, :], bdst, AF.Square,
                                             accum_out=ssq[:, 1, chn:chn + 1])

            # norm scales
            nc.vector.tensor_reduce(ssqt[:, 0:1], ssq[:, 0, :],
                                    axis=mybir.AxisListType.X, op=ALU.add)
            nc.vector.tensor_reduce(ssqt[:, 1:2], ssq[:, 1, :],
                                    axis=mybir.AxisListType.X, op=ALU.add)
            nc.scalar.activation(nrm[:, :], ssqt[:, :], AF.Sqrt)
            nc.vector.tensor_scalar_add(nrm[:, 0:1], nrm[:, 0:1], 1e-5)
            nc.vector.reciprocal(srec[:, :], nrm[:, :])
            nc.vector.tensor_scalar_mul(scl_tn[:, :], srec[:, :], SQRT_N / N)
            nc.vector.tensor_scalar_mul(scl_y[:, :], srec[:, :], SQRT_N)

            # ---- bases (N,M) via PE transposes ----
            for j in range(NCH):
                ns = slice(j * 128, (j + 1) * 128)
                for kc in range(2):
                    tp = pp.tile([128, 128], BF16, tag="tp", bufs=2, name="tp_nm")
                    nc.tensor.transpose(tp[:, :], basesT[:, kc, ns], id_bf[:, :])
                    dst = bases_nm[:, j, kc * 128:(kc + 1) * 128]
                    if (2 * j + kc) % 2 == 0:
                        nc.scalar.copy(dst, tp[:, :])
                    else:
                        nc.vector.tensor_copy(dst, tp[:, :])

            # ---- fc0 matmuls + LN0 ----
            for j in range(NCH):
                ns = slice(j * 128, (j + 1) * 128)
                ps = pp.tile([128, 128], F32, tag="chunk", bufs=3, name="ps_fc0")
                nc.tensor.matmul(ps[:, :], xb[32:36, ns], fc0p[32:36, :],
                                 start=True, stop=True)
                ln_chunk(j, ps[:, :])
                if j % GSZ == GSZ - 1:
                    ln_group(j // GSZ, 0, gelu=False)

        # ================= LN0 + layers + tail (pool pY) =================
        with tc.tile_pool(name="late", bufs=1) as pY:
            t_sb = pY.tile([128, 2, KM, 128], BF16, name="t_sb")

            for li in range(L):
                # hT via PE transposes (f16)
                for j in range(NCH):
                    ns = slice(j * 128, (j + 1) * 128)
                    tp = pp.tile([128, 128], F16, tag="tp", bufs=2, name="tp_h")
                    nc.tensor.transpose(tp[:, :], h[:, j, :], id_f16[:, :])
                    if j % 2 == 0:
                        nc.scalar.copy(hT[:, ns], tp[:, :])
                    else:
                        nc.vector.tensor_copy(hT[:, ns], tp[:, :])
                # xh
                ps_xh = pp.tile([128, 256], F32, tag="mm256", bufs=2, name="ps_xh")
                for j in range(NCH):
                    nc.tensor.matmul(ps_xh[:, :], h[:, j, :], bases_nm[:, j, :],
                                     start=(j == 0), stop=(j == NCH - 1))
                nc.vector.tensor_copy(xh_sb[:, :], ps_xh[:, :])
                # t
                for lc in range(2):
                    for jp in range(KM // 2):
                        ps_t = pp.tile([128, 256], F32, tag="mm256", bufs=2,
                                       name="ps_t")
                        nc.tensor.matmul(ps_t[:, :],
                                         xh_sb[:, lc * 128:(lc + 1) * 128],
                                         spw[:, li, jp * 256:(jp + 1) * 256],
                                         start=True, stop=True)
                        nc.vector.tensor_scalar(
                            t_sb[:, lc, 2 * jp:2 * jp + 2, :], ps_t[:, :],
                            scl_tn[:, lc:lc + 1], None, ALU.mult)
                # y
                ps_y = pp.tile([128, 256], F32, tag="mm256", bufs=2, name="ps_y")
                mmidx = 0
                for lc in range(2):
                    for jj in range(KM):
                        nc.tensor.matmul(ps_y[:, :], t_sb[:, lc, jj, :],
                                         h1t[:, lc, jj, :],
                                         start=(mmidx == 0), stop=(mmidx == 31))
                        mmidx += 1
                nc.scalar.copy(ys_f32[:, :], ps_y[:, :])
                for kc in range(2):
                    tp = pp.tile([128, 128], F32, tag="tp", bufs=2, name="tp_y")
                    nc.tensor.transpose(tp[:, :],
                                        ys_f32[:, kc * 128:(kc + 1) * 128],
                                        id_f32[:, :])
                    nc.vector.tensor_scalar(y_sb[:, kc, :], tp[:, :],
                                            scl_y[:, kc:kc + 1], None, ALU.mult)
                # x1 + x2 per chunk, then LN
                for j in range(NCH):
                    ns = slice(j * 128, (j + 1) * 128)
                    ps = pp.tile([128, 128], F32, tag="chunk", bufs=3, name="ps_x")
                    nc.tensor.matmul(ps[:, :], basesT[:, 0, ns], y_sb[:, 0, :],
                                     start=True, stop=False)
                    nc.tensor.matmul(ps[:, :], basesT[:, 1, ns], y_sb[:, 1, :],
                                     start=False, stop=False)
                    nc.tensor.matmul(ps[:, :], hT[:, ns], wst[:, li, :],
                                     start=False, stop=False)
                    nc.tensor.matmul(ps[:, :], ones_sm[:, 0:128], wsb[:, li, :],
                                     start=False, stop=True)
                    ln_chunk(j, ps[:, :])
                    if j % GSZ == GSZ - 1:
                        ln_group(j // GSZ, li + 1, gelu=(li != L - 1))

            # ---- tail: h3T, fc1+gelu, fc2 ----
            for j in range(NCH):
                ns = slice(j * 128, (j + 1) * 128)
                tp = pp.tile([128, 128], F16, tag="tp", bufs=2, name="tp_h3")
                nc.tensor.transpose(tp[:, :], h[:, j, :], id_f16[:, :])
                if j % 2 == 0:
                    nc.scalar.copy(hT[:, ns], tp[:, :])
                else:
                    nc.vector.tensor_copy(hT[:, ns], tp[:, :])
            for ch in range(16):
                cs = slice(ch * 512, (ch + 1) * 512)
                ps_f = pp.tile([128, 512], F32, tag="wide", bufs=1, name="ps_fc1")
                nc.tensor.matmul(ps_f[:, :], fc1w[:, :], hT[:, cs],
                                 start=True, stop=False)
                nc.tensor.matmul(ps_f[:, :], fc1b[:, :], ones_sm[:, :],
                                 start=False, stop=True)
                h4 = pY.tile([128, 512], F16, tag="h4", bufs=2, name="h4")
                nc.scalar.activation(h4[:, :], ps_f[:, :], AF.Gelu)
                ps_o = pp.tile([1, 512], F32, tag="chunk", bufs=3, name="ps_fc2")
                nc.tensor.matmul(ps_o[:, :], fc2w[:, :], h4[:, :],
                                 start=True, stop=False)
                nc.tensor.matmul(ps_o[:, :], fc2b[:, :], ones_sm[:, :],
                                 start=False, stop=True)
                o_sb = pY.tile([1, 512], F32, tag="o_sb", bufs=2, name="o_sb")
                nc.vector.tensor_copy(o_sb[:, :], ps_o[:, :])
                nc.sync.dma_start(
                    out_d[ch * 512:(ch + 1) * 512].rearrange("(a n) -> a n", a=1),
                    o_sb[:, :])

    nc.finalize()
    return nc


_CACHE = {}


def _get_nc(affine: bool):
    if affine not in _CACHE:
        _CACHE[affine] = _build(affine)
    return _CACHE[affine]


def kernel(**inputs):
    x = np.asarray(inputs["x"], np.float32)
    bw_fourier = np.asarray(inputs["bw_fourier"], np.float32)
    pts_gauss = np.asarray(inputs["pts_gauss"], np.float32)
    bw_gauss = np.asarray(inputs["bw_gauss"], np.float32)
    fc0_w = np.asarray(inputs["fc0_w"], np.float32)
    fc0_b = np.asarray(inputs["fc0_b"], np.float32)
    ln0_g = np.asarray(inputs["ln0_g"], np.float32)
    ln0_b = np.asarray(inputs["ln0_b"], np.float32)
    H1 = np.asarray(inputs["H1"], np.float32)
    sp_weights = np.asarray(inputs["sp_weights"], np.float32)
    ws_w = np.asarray(inputs["ws_w"], np.float32)
    ws_b = np.asarray(inputs["ws_b"], np.float32)
    ln_g = np.asarray(inputs["ln_g"], np.float32)
    ln_b = np.asarray(inputs["ln_b"], np.float32)
    fc1_w = np.asarray(inputs["fc1_w"], np.float32)
    fc1_b = np.asarray(inputs["fc1_b"], np.float32)
    fc2_w = np.asarray(inputs["fc2_w"], np.float32)
    fc2_b = np.asarray(inputs["fc2_b"], np.float32)

    affine = not (
        np.all(ln0_g == 1) and np.all(ln0_b == 0)
        and np.all(ln_g == 1) and np.all(ln_b == 0)
    )
    nc = _get_nc(affine)

    # host packing (layout + tiny param math only)
    a5f = np.zeros((5, 128), np.float32)
    a5f[2, :] = np.concatenate([bw_fourier[:, 0]] * 2)
    a5f[3, :] = np.concatenate([bw_fourier[:, 1]] * 2)
    a5f[4, 0:K1] = np.pi / 2

    w = np.abs(bw_gauss)
    a5g = np.zeros((5, 128), np.float32)
    a5g[0, :] = -w[:, 0]
    a5g[1, :] = -w[:, 1]
    a5g[2, :] = 2 * w[:, 0] * pts_gauss[:, 0]
    a5g[3, :] = 2 * w[:, 1] * pts_gauss[:, 1]
    a5g[4, :] = -(w[:, 0] * pts_gauss[:, 0] ** 2 + w[:, 1] * pts_gauss[:, 1] ** 2)

    fc0p = np.ascontiguousarray(
        np.stack([fc0_w[1], fc0_w[2], fc0_b, fc0_w[0]], 0))

    spw = np.ascontiguousarray(sp_weights.transpose(1, 0, 3, 2)).reshape(
        128, L, KM * 128).astype(bf16)
    h1t = np.ascontiguousarray(
        H1.transpose(2, 0, 1).reshape(2, 128, KM, 256).transpose(1, 0, 2, 3)
    ).astype(bf16)
    wst = np.ascontiguousarray(ws_w.transpose(2, 0, 1)).astype(f16)
    wsb = np.ascontiguousarray(ws_b.reshape(1, L, C)).astype(f16)

    common = dict(
        a5f=a5f, a5g=a5g, fc0p=fc0p, spw=spw, h1t=h1t, wst=wst, wsb=wsb,
        fc1w=fc1_w.astype(f16), fc1b=fc1_b.reshape(1, C).astype(f16),
        fc2w=fc2_w.astype(f16), fc2b=fc2_b.reshape(1, 1).astype(f16),
    )
    if affine:
        lng = np.broadcast_to(
            np.concatenate([ln0_g.reshape(1, C), ln_g], 0)[None], (128, L + 1, C))
        lnb = np.broadcast_to(
            np.concatenate([ln0_b.reshape(1, C), ln_b], 0)[None], (128, L + 1, C))
        common["lng"] = np.ascontiguousarray(lng, np.float32)
        common["lnb"] = np.ascontiguousarray(lnb, np.float32)

    in_maps = []
    for b in range(B):
        xbT = np.ascontiguousarray(
            np.stack([x[b, :, 1], x[b, :, 2], x[b, :, 0],
                      np.ones(N, np.float32)], 0))
        fsq = np.ascontiguousarray(xbT[0:2].reshape(128, 128))
        in_maps.append(dict(common, xb=xbT, fsq=fsq))

    res = run_bass_kernel_spmd(nc, in_maps, core_ids=list(range(B)))
    out = np.stack([np.asarray(res.results[i]["out"]).reshape(N, 1)
                    for i in range(B)], 0)
    return out.astype(np.float32)
